# revision 27
# baseline (speedup 1.0000x reference)
"""Decoder block Bass/Tile kernel for TRN2, SPMD over 8 cores.

Sharding: core c = (batch b = c//4, j = c%4). Each core:
  - computes LN1 + K,V for ALL T_kv tokens of its batch (redundant x4, zero comm)
  - handles 512 queries: chunk A = rows [256j, 256j+256), chunk B = rows
    [256(7-j), 256(7-j)+256)  (causal load balance)
  - attention klen padded to a uniform size (1024 for A, 2048 for B) with
    host-provided -60000 masks so the program is identical on all cores
  - proj + residual + LN2 + MLP + residual for its 512 rows
Host gathers the 8 [512, 1024] shards into the full output.

Layouts: "fm" = [feature(partition), token(free)], "rm" = [token, feature].
LN in rm (bn_stats); rm->fm via PE transposes (identity matmul) into PSUM,
evicted by the Pool engine into flat per-chunk fm tiles. Matmuls fp16 with
fp32 PSUM accumulation. Residual stream fp32.

Engine budget: PE does matmuls+transposes; Scalar does exp/gelu/K-bias;
Pool does fm evictions + attention mask adds + weight DMA issue; DVE does
LN stats/normalize + softmax normalization; SP(sync) does x loads, den
round-trip and output stores.
"""

from contextlib import ExitStack
from dataclasses import dataclass

import numpy as np

import concourse.bass as bass
import concourse.tile as tile
from concourse import masks as cmasks
from concourse import mybir
from concourse._compat import with_exitstack

F32 = mybir.dt.float32
F16 = mybir.dt.float16
MASK_NEG = -60000.0


@dataclass
class Cfg:
    D: int = 1024
    DFF: int = 4096
    H: int = 16  # heads
    DH: int = 64  # head dim
    T_kv: int = 2048
    T_q: int = 512  # 2 chunks of CH
    CH: int = 256
    klenA_pad: int = 1024
    klenB_pad: int = 2048
    mmdt: str = "float16"

    @property
    def HP(self):  # head pairs
        return self.H // 2

    @property
    def VA(self):  # augmented V width (dv + ones column per head)
        return self.H * (self.DH + 1)

    @property
    def NKTA(self):
        return self.klenA_pad // 128

    @property
    def NKTB(self):
        return self.klenB_pad // 128

    @property
    def NMASK(self):  # mask kti-pairs: all of A, masked half of B
        return self.NKTA // 2 + self.NKTB // 4


def _bcast_ap(ap, p=128):
    """[N] dram AP -> [p, N] with partition stride 0."""
    return bass.AP(tensor=ap.tensor, offset=ap.offset, ap=[[0, p]] + list(ap.ap))


@with_exitstack
def decoder_kernel(ctx: ExitStack, tc: tile.TileContext, cfg: Cfg, io: dict):
    nc = tc.nc
    MD = getattr(mybir.dt, cfg.mmdt)
    D, DFF, H, DH = cfg.D, cfg.DFF, cfg.H, cfg.DH
    HP, VA, CH = cfg.HP, cfg.VA, cfg.CH
    T_kv, T_q = cfg.T_kv, cfg.T_q
    ND = D // 128  # feature tiles
    NFF = DFF // 128
    NTKV = T_kv // 128
    NTQ = T_q // 128
    W2 = 2 * CH  # paired-head free width (512)

    # V chunk width for psum (<=512); VA = H*65
    n_vch = (VA + 511) // 512
    while VA % n_vch != 0:
        n_vch += 1
    VCH = VA // n_vch
    assert VCH <= 512

    const = ctx.enter_context(tc.tile_pool(name="const", bufs=1))
    eps_t = const.tile([128, 1], F32)
    nc.vector.memset(eps_t, 1e-5)
    ones_t = const.tile([128, 64], F32)
    nc.vector.memset(ones_t, 1.0)
    ident_t = const.tile([128, 128], MD)
    cmasks.make_identity(nc, ident_t)
    bq_sb = const.tile([128, ND], F32)
    nc.gpsimd.dma_start(out=bq_sb, in_=io["bq"].rearrange("(t p) -> p t", p=128))
    bk_sb = const.tile([128, ND], F32)
    nc.gpsimd.dma_start(out=bk_sb, in_=io["bk"].rearrange("(t p) -> p t", p=128))
    bfc1_sb = const.tile([128, NFF], F32)
    nc.gpsimd.dma_start(out=bfc1_sb, in_=io["bfc1"].rearrange("(t p) -> p t", p=128))
    vb_sb = const.tile([128, VA], F32)
    nc.gpsimd.dma_start(out=vb_sb, in_=_bcast_ap(io["vb"]))

    # ---------------- LN + PE-transpose helper ----------------
    def ln_transpose(src_dram, src_sb, n_tiles, fm_flat, pool, stats, tpp, tagp):
        """LN rows rt, then transpose into fm_flat[c][:, d*512 + t]."""
        for rt in range(n_tiles):
            if src_dram is not None:
                x_t = pool.tile([128, D], F32, tag="ln_in", name=f"{tagp}_in")
                nc.sync.dma_start(
                    out=x_t, in_=src_dram[rt * 128 : (rt + 1) * 128, :]
                )
            else:
                x_t = src_sb[rt]
            nsub = D // 512
            st = stats.tile([128, nsub, 6], F32, tag="ln_st")
            for s in range(nsub):
                nc.vector.bn_stats(
                    out=st[:, s, :], in_=x_t[:, s * 512 : (s + 1) * 512]
                )
            mv = stats.tile([128, 2], F32, tag="ln_mv")
            nc.vector.bn_aggr(out=mv, in_=st)
            sd = stats.tile([128, 1], F32, tag="ln_sd")
            nc.scalar.activation(
                out=sd, in_=mv[:, 1:2],
                func=mybir.ActivationFunctionType.Sqrt, bias=eps_t,
            )
            rec = stats.tile([128, 1], F32, tag="ln_rec")
            nc.vector.reciprocal(out=rec, in_=sd)
            xh = pool.tile([128, D], MD, tag="ln_xh", name=f"{tagp}_xh")
            nc.vector.tensor_scalar(
                out=xh, in0=x_t, scalar1=mv[:, 0:1], scalar2=rec,
                op0=mybir.AluOpType.subtract, op1=mybir.AluOpType.mult,
            )
            c, j = rt // 4, rt % 4
            fmt = fm_flat[c].rearrange("p (d t) -> p d t", d=ND)
            for half in range(2):
                pst = tpp.tile([128, 512], MD, tag="tp", name=f"tp_{tagp}")
                for d4 in range(4):
                    d = half * 4 + d4
                    nc.tensor.matmul(
                        pst[:, d4 * 128 : (d4 + 1) * 128],
                        xh[:, d * 128 : (d + 1) * 128],
                        ident_t,
                        is_transpose=True,
                        start=True, stop=True,
                    )
                dst = fmt[:, half * 4 : half * 4 + 4, j * 128 : (j + 1) * 128]
                nc.scalar.copy(out=dst, in_=pst)

    if True:
        acts = ctx.enter_context(tc.tile_pool(name="acts", bufs=1))
        K_sb = [acts.tile([128, T_kv], MD, tag=f"K{d}", name=f"K{d}") for d in range(ND)]
        Q_sb = [acts.tile([128, 2 * T_q], MD, tag=f"Q{d}", name=f"Q{d}") for d in range(ND)]
        for d in range(ND):
            nc.vector.memset(Q_sb[d], 0.0)
        V_sb = [acts.tile([128, VA], MD, tag=f"V{t}", name=f"V{t}") for t in range(NTKV)]
        O_sb = [acts.tile([128, T_q], MD, tag=f"O{h}", name=f"O{h}") for h in range(HP)]

        with tc.tile_pool(name="fm", bufs=1) as fmp:
            NCHKV = T_kv // 512
            xkv_fm = [
                fmp.tile([128, ND * 512], MD, tag=f"xkvfm{c}", name=f"xkvfm{c}")
                for c in range(NCHKV)
            ]
            xq_fm = [fmp.tile([128, ND * 512], MD, tag="xqfm", name="xqfm")]
            # ---------------- QKV projections ----------------
            with tc.tile_pool(name="wqk", bufs=3) as wqk, tc.tile_pool(
                name="wv", bufs=1
            ) as wvp, tc.tile_pool(name="psqkv", bufs=4, space="PSUM") as psq, \
                tc.tile_pool(name="tp1", bufs=2, space="PSUM") as tpp:
                wv_sb = [wvp.tile([128, VA], MD, tag=f"wv{kt}", name=f"wv{kt}") for kt in range(ND)]
                for kt in range(ND):
                    nc.gpsimd.dma_start(
                        out=wv_sb[kt], in_=io["wv"][kt * 128 : (kt + 1) * 128, :]
                    )
                with tc.tile_pool(name="ln1", bufs=3) as lnp, tc.tile_pool(
                    name="ln1st", bufs=4
                ) as lnst:
                    ln_transpose(io["x_kv"], None, NTKV, xkv_fm, lnp, lnst, tpp, "kv")
                    ln_transpose(io["x_q"], None, NTQ, xq_fm, lnp, lnst, tpp, "q")

                # V (consumes fm tiles in LN emission order)
                for tt in range(NTKV):
                    for ch in range(n_vch):
                        ps = psq.tile([128, VCH], F32, tag="psv", bufs=3)
                        for kt in range(ND):
                            nc.tensor.matmul(
                                ps,
                                xkv_fm[tt // 4][
                                    :, kt * 512 + (tt % 4) * 128 : kt * 512 + (tt % 4 + 1) * 128
                                ],
                                wv_sb[kt][:, ch * VCH : (ch + 1) * VCH],
                                start=(kt == 0),
                                stop=(kt == ND - 1),
                            )
                        nc.vector.tensor_add(
                            out=V_sb[tt][:, ch * VCH : (ch + 1) * VCH],
                            in0=ps,
                            in1=vb_sb[:, ch * VCH : (ch + 1) * VCH],
                        )
                # K then Q (weights stationary, fm out)
                for which, wname, bias_sb, fm_src, out_sb, T in (
                    ("k", "wk", bk_sb, xkv_fm, K_sb, T_kv),
                    ("q", "wq", bq_sb, xq_fm, Q_sb, T_q),
                ):
                    for do in range(ND):
                        wb = wqk.tile([128, ND, 128], MD, tag="wqk")
                        nc.gpsimd.dma_start(
                            out=wb,
                            in_=io[wname][:, do * 128 : (do + 1) * 128].rearrange(
                                "(kt p) c -> p kt c", p=128
                            ),
                        )
                        for tch in range(T // 512):
                            ps = psq.tile([128, 512], F32, tag="psqk", bufs=3)
                            for kt in range(ND):
                                nc.tensor.matmul(
                                    ps,
                                    wb[:, kt, :],
                                    fm_src[tch][:, kt * 512 : (kt + 1) * 512],
                                    start=(kt == 0),
                                    stop=(kt == ND - 1),
                                )
                            if which == "k":
                                nc.scalar.activation(
                                    out=out_sb[do][:, tch * 512 : (tch + 1) * 512],
                                    in_=ps,
                                    func=mybir.ActivationFunctionType.Identity,
                                    bias=bias_sb[:, do : do + 1],
                                )
                            else:
                                # Q: scatter into per-(chunk, head) blocks with the
                                # complementary head's partitions left zero
                                for ci in range(2):
                                    for h in range(2):
                                        blk = (2 * ci + h) * CH
                                        nc.scalar.activation(
                                            out=out_sb[do][
                                                h * 64 : (h + 1) * 64,
                                                blk : blk + CH,
                                            ],
                                            in_=ps[
                                                h * 64 : (h + 1) * 64,
                                                ci * CH : (ci + 1) * CH,
                                            ],
                                            func=mybir.ActivationFunctionType.Identity,
                                            bias=bias_sb[h * 64 : (h + 1) * 64, do : do + 1],
                                        )

        # ---------------- persistent mid tiles (alloc after fm frees) ----
        mid = ctx.enter_context(tc.tile_pool(name="mid", bufs=1))
        x2_sb = [mid.tile([128, D], F32, tag=f"x2_{t}", name=f"x2_{t}")
                 for t in range(NTQ)]
        xq2_fm = mid.tile([128, ND * T_q], MD, tag="xq2fm", name="xq2fm")

        # ---------------- attention + proj ----------------
        chunks = [(0, cfg.NKTA, 0), (1, cfg.NKTB, None)]
        with tc.tile_pool(name="attn_w", bufs=1) as awp:
            # prefetch wproj while attention runs
            wproj_sb = [awp.tile([128, D], MD, tag=f"wp{d}", name=f"wp{d}") for d in range(ND)]
            for d in range(ND):
                nc.gpsimd.dma_start(
                    out=wproj_sb[d], in_=io["wproj"][d * 128 : (d + 1) * 128, :]
                )
            with tc.tile_pool(name="attn_m", bufs=1) as mp, tc.tile_pool(
                name="attn_p", bufs=4
            ) as pp, tc.tile_pool(name="attn_ps", bufs=2, space="PSUM"
            ) as aps, tc.tile_pool(name="attn_po", bufs=4, space="PSUM"
            ) as ops, tc.tile_pool(name="attn_rd", bufs=1, space="DRAM") as rdp:
                rsd = rdp.tile([2 * HP, W2], F32, tag="rsd", name="rsd")
                # masks: kti-pairs [128, 1024]; chunk A pairs 0..NKTA/2-1,
                # chunk B masked pairs at NKTA/2 ..
                mask_sb = []
                for k in range(cfg.NMASK):
                    m = mp.tile([128, 2 * W2], MD, tag=f"mask{k}")
                    nc.gpsimd.dma_start(out=m, in_=io["masks"][k, :, :])
                    mask_sb.append(m)
                for ci, nkt, _ in chunks:
                    cc = slice(ci * CH, (ci + 1) * CH)
                    npair = nkt // 2
                    for hp in range(HP):
                        po = [ops.tile([128, CH], F32, tag="po", name="po", bufs=4)
                              for _ in range(2)]
                        for kp in range(npair):
                            ps2 = aps.tile([128, 2 * W2], F32, tag="ps_s")
                            for sub in range(2):
                                kti = 2 * kp + sub
                                nc.tensor.matmul(
                                    ps2[:, sub * W2 : (sub + 1) * W2],
                                    K_sb[hp][:, kti * 128 : (kti + 1) * 128],
                                    Q_sb[hp][:, 2 * ci * CH : 2 * ci * CH + W2],
                                    start=True, stop=True,
                                )
                            if ci == 0:
                                nc.vector.tensor_add(out=ps2, in0=ps2, in1=mask_sb[kp])
                            elif 2 * kp * 128 >= cfg.klenB_pad // 2:
                                mi = cfg.NKTA // 2 + kp - cfg.NKTB // 4
                                nc.vector.tensor_add(out=ps2, in0=ps2, in1=mask_sb[mi])
                            pt = pp.tile([128, 2 * W2], MD, tag="pt")
                            nc.scalar.activation(
                                out=pt, in_=ps2,
                                func=mybir.ActivationFunctionType.Exp,
                            )
                            for sub in range(2):
                                kti = 2 * kp + sub
                                for h in range(2):
                                    hg = 2 * hp + h
                                    nc.tensor.matmul(
                                        po[h][0:65, :],
                                        V_sb[kti][:, hg * 65 : hg * 65 + 65],
                                        pt[:, sub * W2 + h * CH : sub * W2 + (h + 1) * CH],
                                        start=(kti == 0),
                                        stop=(kti == nkt - 1),
                                    )
                        # normalize + evict: den rows -> partition 0,
                        # reciprocal there, DMA-broadcast across partitions,
                        # then multiply the numerators
                        den = pp.tile([128, W2], F32, tag="densb")
                        for h in range(2):
                            nc.scalar.copy(
                                out=den[0:1, h * CH : (h + 1) * CH],
                                in_=po[h][64:65, :],
                            )
                        rc1 = pp.tile([128, W2], F32, tag="rc1sb")
                        nc.vector.reciprocal(out=rc1[0:1, :], in_=den[0:1, :])
                        slot = ci * HP + hp
                        nc.sync.dma_start(
                            out=rsd[slot : slot + 1, :], in_=rc1[0:1, :]
                        )
                        rc = pp.tile([128, W2], F32, tag="rcsb")
                        nc.sync.dma_start(
                            out=rc,
                            in_=bass.AP(
                                tensor=rsd.tensor,
                                offset=rsd.offset + slot * W2,
                                ap=[[0, 128], [1, W2]],
                            ),
                        )
                        for h in range(2):
                            nc.vector.tensor_mul(
                                out=O_sb[hp][h * 64 : (h + 1) * 64, cc],
                                in0=po[h][0:64, :],
                                in1=rc[h * 64 : (h + 1) * 64, h * CH : (h + 1) * CH],
                            )

            # ---------------- proj + residual ----------------
            with tc.tile_pool(name="proj", bufs=3) as prp, tc.tile_pool(
                name="projps", bufs=4, space="PSUM"
            ) as prps:
                for qt in range(NTQ):
                    x_t = prp.tile([128, D], F32, tag="xq_res")
                    nc.sync.dma_start(
                        out=x_t, in_=io["x_q"][qt * 128 : (qt + 1) * 128, :]
                    )
                    for ch2 in range(D // 512):
                        ps = prps.tile([128, 512], F32, tag="pspr")
                        for hp in range(ND):
                            nc.tensor.matmul(
                                ps,
                                O_sb[hp][:, qt * 128 : (qt + 1) * 128],
                                wproj_sb[hp][:, ch2 * 512 : (ch2 + 1) * 512],
                                start=(hp == 0),
                                stop=(hp == ND - 1),
                            )
                        nc.vector.tensor_add(
                            out=x2_sb[qt][:, ch2 * 512 : (ch2 + 1) * 512],
                            in0=ps,
                            in1=x_t[:, ch2 * 512 : (ch2 + 1) * 512],
                        )

            # ---------------- LN2 + transpose (overlaps proj tail) --------
            with tc.tile_pool(name="ln2", bufs=3) as ln2p, tc.tile_pool(
                name="ln2st", bufs=4
            ) as ln2st, tc.tile_pool(name="tp2", bufs=2, space="PSUM") as tpp2:
                ln_transpose(None, x2_sb, NTQ, [xq2_fm], ln2p, ln2st, tpp2, "l2")

    # ---------------- fc1 + gelu + fc2 (pipelined) ----------------
    ghp = ctx.enter_context(tc.tile_pool(name="gh", bufs=1))
    gh_sb = [ghp.tile([128, T_q], MD, tag=f"gh{f}", name=f"gh{f}") for f in range(NFF)]
    with tc.tile_pool(name="fc1w", bufs=4) as f1w, tc.tile_pool(
        name="fc2w", bufs=4
    ) as f2w, tc.tile_pool(name="fc2wB", bufs=1) as f2wB, tc.tile_pool(
        name="fc2out", bufs=3
    ) as f2o, tc.tile_pool(name="fcps", bufs=3, space="PSUM"
    ) as fps, tc.tile_pool(name="fc2acc", bufs=1, space="PSUM") as f2ps:
        # preload most of sweep-1's fc2 weights during sweep 0
        NPRE = 24
        wb2_s1 = [f2wB.tile([128, 512], MD, tag=f"wfc2b{ff}", name=f"wfc2b{ff}")
                  for ff in range(NPRE)]
        for sweep in range(2):
            accs = {}
            for qt in range(NTQ):
                accs[qt] = f2ps.tile(
                    [128, 512], F32, tag=f"acc{qt}", name=f"acc{qt}"
                )
            for ff in range(NFF):
                if sweep == 0:
                    wb = f1w.tile([128, ND, 128], MD, tag="wfc1")
                    nc.gpsimd.dma_start(
                        out=wb,
                        in_=io["wfc1"][:, ff * 128 : (ff + 1) * 128].rearrange(
                            "(kt p) c -> p kt c", p=128
                        ),
                    )
                    ps = fps.tile([128, T_q], F32, tag="psf1")
                    for kt in range(ND):
                        nc.tensor.matmul(
                            ps, wb[:, kt, :],
                            xq2_fm[:, kt * 512 : (kt + 1) * 512],
                            start=(kt == 0), stop=(kt == ND - 1),
                        )
                    nc.scalar.activation(
                        out=gh_sb[ff], in_=ps,
                        func=mybir.ActivationFunctionType.Gelu,
                        bias=bfc1_sb[:, ff : ff + 1],
                    )
                if sweep == 0:
                    wb2 = f2w.tile([128, 512], MD, tag="wfc2")
                    nc.gpsimd.dma_start(
                        out=wb2, in_=io["wfc2"][ff * 128 : (ff + 1) * 128, 0:512]
                    )
                    # interleave sweep-1 weight prefetch on the same queue
                    if ff < NPRE:
                        nc.gpsimd.dma_start(
                            out=wb2_s1[ff],
                            in_=io["wfc2"][ff * 128 : (ff + 1) * 128, 512:1024],
                        )
                elif ff < NPRE:
                    wb2 = wb2_s1[ff]
                else:
                    wb2 = f2w.tile([128, 512], MD, tag="wfc2")
                    nc.gpsimd.dma_start(
                        out=wb2, in_=io["wfc2"][ff * 128 : (ff + 1) * 128, 512:1024]
                    )
                for qt in range(NTQ):
                    nc.tensor.matmul(
                        accs[qt],
                        gh_sb[ff][:, qt * 128 : (qt + 1) * 128],
                        wb2,
                        start=(ff == 0),
                        stop=(ff == NFF - 1),
                    )
            for qt in range(NTQ):
                o = f2o.tile([128, 512], F32, tag="osb")
                nc.vector.tensor_add(
                    out=o,
                    in0=accs[qt],
                    in1=x2_sb[qt][:, sweep * 512 : (sweep + 1) * 512],
                )
                nc.sync.dma_start(
                    out=io["out"][
                        qt * 128 : (qt + 1) * 128,
                        sweep * 512 : (sweep + 1) * 512,
                    ],
                    in_=o,
                )


def split_drain_waits(nc):
    """walrus CoreV3 rejects >1 sync wait on several instruction types;
    split extras into single-wait NOPs preceding the instruction on the
    same (in-order) engine."""
    idx = 0

    def fix_block(b):
        nonlocal idx
        new = []
        changed = False
        for inst in b.instructions:
            si = inst.sync_info
            if si is not None and si.on_wait and len(si.on_wait) > 1:
                waits = list(si.on_wait)
                for w in waits[:-1]:
                    idx += 1
                    nop = mybir.InstNoOp(
                        name=f"I-dsplit-{idx}",
                        sync_info=mybir.SyncInfo(on_wait=[w], on_update=[]),
                    )
                    nop.engine = inst.engine
                    new.append(nop)
                inst.sync_info = mybir.SyncInfo(
                    on_wait=[waits[-1]], on_update=list(si.on_update or [])
                )
                changed = True
            new.append(inst)
        if changed:
            b.instructions = new

    for f in nc.m.functions:
        for b in f.blocks:
            fix_block(b)


def declare_io(nc, cfg: Cfg):
    c = cfg
    WD = getattr(mybir.dt, c.mmdt)
    spec = {
        "x_kv": ([c.T_kv, c.D], F32, False),
        "x_q": ([c.T_q, c.D], F32, False),
        "wq": ([c.D, c.D], WD, False),
        "wk": ([c.D, c.D], WD, False),
        "wv": ([c.D, c.VA], WD, False),
        "bq": ([c.D], F32, False),
        "bk": ([c.D], F32, False),
        "vb": ([c.VA], F32, False),
        "wproj": ([c.D, c.D], WD, False),
        "wfc1": ([c.D, c.DFF], WD, False),
        "bfc1": ([c.DFF], F32, False),
        "wfc2": ([c.DFF, c.D], WD, False),
        "masks": ([c.NMASK, 128, 4 * c.CH], WD, False),
        "out": ([c.T_q, c.D], F32, True),
    }
    io = {}
    for name, (shape, dt, is_out) in spec.items():
        io[name] = nc.declare_dram_parameter(name, shape, dt, isOutput=is_out).ap()
    return io


def build(cfg: Cfg, split: bool = True):
    nc = bass.Bass(num_devices=8)
    io = declare_io(nc, cfg)
    with tile.TileContext(nc) as tc:
        decoder_kernel(tc, cfg, io)
    if split:
        split_drain_waits(nc)
    return nc


# ======================= host-side prep =======================


def make_masks(cfg: Cfg, qgA, qgB):
    """[NMASK, 128, 1024] kti-pair masks: 0 where key k <= query q (valid),
    else -60000. Layout per pair: [kti_even(h0 256|h1 256) | kti_odd(...)].
    Pairs 0..NKTA/2-1 are chunk A kti (0..NKTA-1); the rest are chunk B's
    masked upper-half kti (NKTB/2..NKTB-1)."""
    m = np.full((cfg.NMASK, 128, 2 * cfg.CH, 2), MASK_NEG, np.float32)

    def blk(qg, kti):
        q = qg + np.arange(cfg.CH)[None, :]
        kg = kti * 128 + np.arange(128)[:, None]
        return (kg > q).astype(np.float32) * MASK_NEG

    for kp in range(cfg.NKTA // 2):
        for sub in range(2):
            b = blk(qgA, 2 * kp + sub)
            m[kp, :, 0 : cfg.CH, sub] = b
            m[kp, :, cfg.CH : 2 * cfg.CH, sub] = b
    for i, kp in enumerate(range(cfg.NKTB // 4, cfg.NKTB // 2)):
        for sub in range(2):
            b = blk(qgB, 2 * kp + sub)
            mi = cfg.NKTA // 2 + i
            m[mi, :, 0 : cfg.CH, sub] = b
            m[mi, :, cfg.CH : 2 * cfg.CH, sub] = b
    # [NMASK,128,512,2] -> [NMASK,128,1024] with pair-sub major order
    m = m.transpose(0, 1, 3, 2).reshape(cfg.NMASK, 128, 4 * cfg.CH)
    return m.astype(np.float16)


def host_prep(cfg: Cfg, x, ln1_g, ln1_b, w_qkv, w_proj, ln2_g, ln2_b, w_fc1, w_fc2):
    """Returns (in_maps list of 8 dicts, assemble(results)->full out)."""
    D, H, DH = cfg.D, cfg.H, cfg.DH
    x = np.asarray(x, np.float32)
    B = x.shape[0]
    w_qkv = np.asarray(w_qkv, np.float32)
    bqkv = np.asarray(ln1_b, np.float32) @ w_qkv  # [3D]
    w_qkv = w_qkv * np.asarray(ln1_g, np.float32)[:, None]
    bq = bqkv[0:D] / np.sqrt(DH).astype(np.float32)
    bk = bqkv[D : 2 * D]
    bv = bqkv[2 * D : 3 * D]
    wq = w_qkv[:, 0:D] / np.sqrt(DH).astype(np.float32)
    wk = w_qkv[:, D : 2 * D]
    wv = w_qkv[:, 2 * D : 3 * D]
    wv_aug = np.zeros((D, cfg.VA), np.float32)
    vb_aug = np.zeros((cfg.VA,), np.float32)
    for h in range(H):
        wv_aug[:, h * (DH + 1) : h * (DH + 1) + DH] = wv[:, h * DH : (h + 1) * DH]
        vb_aug[h * (DH + 1) : h * (DH + 1) + DH] = bv[h * DH : (h + 1) * DH]
        vb_aug[h * (DH + 1) + DH] = 1.0
    bfc1 = np.asarray(ln2_b, np.float32) @ np.asarray(w_fc1, np.float32)
    wfc1 = np.asarray(w_fc1, np.float32) * np.asarray(ln2_g, np.float32)[:, None]

    wd = np.float32 if cfg.mmdt == "float32" else np.float16
    weights = {
        "wq": wq.astype(wd),
        "wk": wk.astype(wd),
        "wv": wv_aug.astype(wd),
        "bq": bq.astype(np.float32),
        "bk": bk.astype(np.float32),
        "vb": vb_aug.astype(np.float32),
        "wproj": np.asarray(w_proj, np.float32).astype(wd),
        "wfc1": wfc1.astype(wd),
        "bfc1": bfc1.astype(np.float32),
        "wfc2": np.asarray(w_fc2, np.float32).astype(wd),
    }

    in_maps = []
    core_rows = []
    n_j = 4  # chunk pairs per batch
    for c in range(8):
        b, j = c // n_j, c % n_j
        qgA, qgB = cfg.CH * j, cfg.CH * (2 * n_j - 1 - j)
        rows = np.r_[qgA : qgA + cfg.CH, qgB : qgB + cfg.CH]
        core_rows.append((b, rows))
        im = dict(weights)
        im["x_kv"] = np.ascontiguousarray(x[b])
        im["x_q"] = np.ascontiguousarray(x[b][rows])
        im["masks"] = make_masks(cfg, qgA, qgB).astype(wd)
        in_maps.append(im)

    def assemble(results):
        out = np.zeros((B, x.shape[1], D), np.float32)
        for c, (b, rows) in enumerate(core_rows):
            out[b][rows] = results[c]["out"]
        return out

    return in_maps, assemble


# ======================= public entry point =======================

LAST_RESULTS = {}
_CACHE = {}


def kernel(x, ln1_g, ln1_b, w_qkv, w_proj, ln2_g, ln2_b, w_fc1, w_fc2,
           _trace=False):
    """Full-input decoder block on 8 TRN2 NeuronCores; returns full output."""
    from concourse.bass_utils import run_bass_kernel_spmd

    cfg = Cfg()
    in_maps, assemble = host_prep(
        cfg, x, ln1_g, ln1_b, w_qkv, w_proj, ln2_g, ln2_b, w_fc1, w_fc2
    )
    if "nc" not in _CACHE:
        _CACHE["nc"] = build(cfg)
    res = run_bass_kernel_spmd(
        _CACHE["nc"], in_maps, core_ids=list(range(8)), trace=_trace
    )
    LAST_RESULTS["res"] = res
    return assemble(res.results)


# revision 28
# speedup vs baseline: 1.1388x; 1.1388x over previous
"""Decoder block Bass/Tile kernel for TRN2, SPMD over 8 cores.

Sharding: core c = (batch b = c//4, j = c%4). Each core:
  - computes LN1 + K,V for ALL T_kv tokens of its batch (redundant x4, zero comm)
  - handles 512 queries: chunk A = rows [256j, 256j+256), chunk B = rows
    [256(7-j), 256(7-j)+256)  (causal load balance)
  - attention klen padded to a uniform size (1024 for A, 2048 for B) with
    host-provided -60000 masks so the program is identical on all cores
  - proj + residual + LN2 + MLP + residual for its 512 rows
Host gathers the 8 [512, 1024] shards into the full output.

Layouts: "fm" = [feature(partition), token(free)], "rm" = [token, feature].
LN in rm (bn_stats); rm->fm via PE transposes (identity matmul) into PSUM,
evicted by the Pool engine into flat per-chunk fm tiles. Matmuls fp16 with
fp32 PSUM accumulation. Residual stream fp32.

Engine budget: PE does matmuls+transposes; Scalar does exp/gelu/K-bias;
Pool does fm evictions + attention mask adds + weight DMA issue; DVE does
LN stats/normalize + softmax normalization; SP(sync) does x loads, den
round-trip and output stores.
"""

from contextlib import ExitStack
from dataclasses import dataclass

import numpy as np

import concourse.bass as bass
import concourse.tile as tile
from concourse import masks as cmasks
from concourse import mybir
from concourse._compat import with_exitstack

F32 = mybir.dt.float32
F16 = mybir.dt.float16
MASK_NEG = -60000.0


@dataclass
class Cfg:
    D: int = 1024
    DFF: int = 4096
    H: int = 16  # heads
    DH: int = 64  # head dim
    T_kv: int = 2048
    T_q: int = 512  # 2 chunks of CH
    CH: int = 256
    klenA_pad: int = 1024
    klenB_pad: int = 2048
    mmdt: str = "float16"

    @property
    def HP(self):  # head pairs
        return self.H // 2

    @property
    def VA(self):  # augmented V width (dv + ones column per head)
        return self.H * (self.DH + 1)

    @property
    def NKTA(self):
        return self.klenA_pad // 128

    @property
    def NKTB(self):
        return self.klenB_pad // 128

    @property
    def NMASK(self):  # mask kti-pairs: all of A, masked half of B
        return self.NKTA // 2 + self.NKTB // 4


def _bcast_ap(ap, p=128):
    """[N] dram AP -> [p, N] with partition stride 0."""
    return bass.AP(tensor=ap.tensor, offset=ap.offset, ap=[[0, p]] + list(ap.ap))


@with_exitstack
def decoder_kernel(ctx: ExitStack, tc: tile.TileContext, cfg: Cfg, io: dict):
    nc = tc.nc
    MD = getattr(mybir.dt, cfg.mmdt)
    D, DFF, H, DH = cfg.D, cfg.DFF, cfg.H, cfg.DH
    HP, VA, CH = cfg.HP, cfg.VA, cfg.CH
    T_kv, T_q = cfg.T_kv, cfg.T_q
    ND = D // 128  # feature tiles
    NFF = DFF // 128
    NTKV = T_kv // 128
    NTQ = T_q // 128
    W2 = 2 * CH  # paired-head free width (512)

    # V chunk width for psum (<=512); VA = H*65
    n_vch = (VA + 511) // 512
    while VA % n_vch != 0:
        n_vch += 1
    VCH = VA // n_vch
    assert VCH <= 512

    const = ctx.enter_context(tc.tile_pool(name="const", bufs=1))
    eps_t = const.tile([128, 1], F32)
    nc.vector.memset(eps_t, 1e-5)
    ones_t = const.tile([128, 64], F32)
    nc.vector.memset(ones_t, 1.0)
    ident_t = const.tile([128, 128], MD)
    cmasks.make_identity(nc, ident_t)
    bq_sb = const.tile([128, ND], F32)
    nc.gpsimd.dma_start(out=bq_sb, in_=io["bq"].rearrange("(t p) -> p t", p=128))
    bk_sb = const.tile([128, ND], F32)
    nc.gpsimd.dma_start(out=bk_sb, in_=io["bk"].rearrange("(t p) -> p t", p=128))
    bfc1_sb = const.tile([128, NFF], F32)
    nc.gpsimd.dma_start(out=bfc1_sb, in_=io["bfc1"].rearrange("(t p) -> p t", p=128))
    vb_sb = const.tile([128, VA], F32)
    nc.gpsimd.dma_start(out=vb_sb, in_=_bcast_ap(io["vb"]))

    # ---------------- LN + PE-transpose helper ----------------
    def ln_transpose(src_dram, src_sb, n_tiles, fm_flat, pool, stats, tpp, tagp):
        """LN rows rt, then transpose into fm_flat[c][:, d*512 + t]."""
        for rt in range(n_tiles):
            if src_dram is not None:
                x_t = pool.tile([128, D], F32, tag="ln_in", name=f"{tagp}_in")
                nc.sync.dma_start(
                    out=x_t, in_=src_dram[rt * 128 : (rt + 1) * 128, :]
                )
            else:
                x_t = src_sb[rt]
            nsub = D // 512
            st = stats.tile([128, nsub, 6], F32, tag="ln_st")
            for s in range(nsub):
                nc.vector.bn_stats(
                    out=st[:, s, :], in_=x_t[:, s * 512 : (s + 1) * 512]
                )
            mv = stats.tile([128, 2], F32, tag="ln_mv")
            nc.vector.bn_aggr(out=mv, in_=st)
            sd = stats.tile([128, 1], F32, tag="ln_sd")
            nc.scalar.activation(
                out=sd, in_=mv[:, 1:2],
                func=mybir.ActivationFunctionType.Sqrt, bias=eps_t,
            )
            rec = stats.tile([128, 1], F32, tag="ln_rec")
            nc.vector.reciprocal(out=rec, in_=sd)
            xh = pool.tile([128, D], MD, tag="ln_xh", name=f"{tagp}_xh")
            nc.vector.tensor_scalar(
                out=xh, in0=x_t, scalar1=mv[:, 0:1], scalar2=rec,
                op0=mybir.AluOpType.subtract, op1=mybir.AluOpType.mult,
            )
            c, j = rt // 4, rt % 4
            fmt = fm_flat[c].rearrange("p (d t) -> p d t", d=ND)
            for half in range(2):
                pst = tpp.tile([128, 512], MD, tag="tp", name=f"tp_{tagp}")
                for d4 in range(4):
                    d = half * 4 + d4
                    nc.tensor.matmul(
                        pst[:, d4 * 128 : (d4 + 1) * 128],
                        xh[:, d * 128 : (d + 1) * 128],
                        ident_t,
                        is_transpose=True,
                        start=True, stop=True,
                    )
                dst = fmt[:, half * 4 : half * 4 + 4, j * 128 : (j + 1) * 128]
                nc.scalar.copy(out=dst, in_=pst)

    if True:
        acts = ctx.enter_context(tc.tile_pool(name="acts", bufs=1))
        K_sb = [acts.tile([128, T_kv], MD, tag=f"K{d}", name=f"K{d}") for d in range(ND)]
        Q_sb = [acts.tile([128, 2 * T_q], MD, tag=f"Q{d}", name=f"Q{d}") for d in range(ND)]
        for d in range(ND):
            nc.vector.memset(Q_sb[d], 0.0)
        V_sb = [acts.tile([128, VA], MD, tag=f"V{t}", name=f"V{t}") for t in range(NTKV)]
        O_sb = [acts.tile([128, T_q], MD, tag=f"O{h}", name=f"O{h}") for h in range(HP)]

        with tc.tile_pool(name="fm", bufs=1) as fmp:
            NCHKV = T_kv // 512
            xkv_fm = [
                fmp.tile([128, ND * 512], MD, tag=f"xkvfm{c}", name=f"xkvfm{c}")
                for c in range(NCHKV)
            ]
            xq_fm = [fmp.tile([128, ND * 512], MD, tag="xqfm", name="xqfm")]
            # ---------------- QKV projections ----------------
            with tc.tile_pool(name="wqk", bufs=3) as wqk, tc.tile_pool(
                name="wv", bufs=1
            ) as wvp, tc.tile_pool(name="psqkv", bufs=4, space="PSUM") as psq, \
                tc.tile_pool(name="tp1", bufs=2, space="PSUM") as tpp:
                wv_sb = [wvp.tile([128, VA], MD, tag=f"wv{kt}", name=f"wv{kt}") for kt in range(ND)]
                for kt in range(ND):
                    nc.gpsimd.dma_start(
                        out=wv_sb[kt], in_=io["wv"][kt * 128 : (kt + 1) * 128, :]
                    )
                with tc.tile_pool(name="ln1", bufs=3) as lnp, tc.tile_pool(
                    name="ln1st", bufs=4
                ) as lnst:
                    ln_transpose(io["x_kv"], None, NTKV, xkv_fm, lnp, lnst, tpp, "kv")
                    ln_transpose(io["x_q"], None, NTQ, xq_fm, lnp, lnst, tpp, "q")

                # V (consumes fm tiles in LN emission order)
                for tt in range(NTKV):
                    for ch in range(n_vch):
                        ps = psq.tile([128, VCH], F32, tag="psv", bufs=3)
                        for kt in range(ND):
                            nc.tensor.matmul(
                                ps,
                                xkv_fm[tt // 4][
                                    :, kt * 512 + (tt % 4) * 128 : kt * 512 + (tt % 4 + 1) * 128
                                ],
                                wv_sb[kt][:, ch * VCH : (ch + 1) * VCH],
                                start=(kt == 0),
                                stop=(kt == ND - 1),
                            )
                        nc.vector.tensor_add(
                            out=V_sb[tt][:, ch * VCH : (ch + 1) * VCH],
                            in0=ps,
                            in1=vb_sb[:, ch * VCH : (ch + 1) * VCH],
                        )
                # K then Q (weights stationary, fm out)
                for which, wname, bias_sb, fm_src, out_sb, T in (
                    ("k", "wk", bk_sb, xkv_fm, K_sb, T_kv),
                    ("q", "wq", bq_sb, xq_fm, Q_sb, T_q),
                ):
                    for do in range(ND):
                        wb = wqk.tile([128, ND, 128], MD, tag="wqk")
                        nc.gpsimd.dma_start(
                            out=wb,
                            in_=io[wname][:, do * 128 : (do + 1) * 128].rearrange(
                                "(kt p) c -> p kt c", p=128
                            ),
                        )
                        for tch in range(T // 512):
                            ps = psq.tile([128, 512], F32, tag="psqk", bufs=3)
                            for kt in range(ND):
                                nc.tensor.matmul(
                                    ps,
                                    wb[:, kt, :],
                                    fm_src[tch][:, kt * 512 : (kt + 1) * 512],
                                    start=(kt == 0),
                                    stop=(kt == ND - 1),
                                )
                            if which == "k":
                                nc.scalar.activation(
                                    out=out_sb[do][:, tch * 512 : (tch + 1) * 512],
                                    in_=ps,
                                    func=mybir.ActivationFunctionType.Identity,
                                    bias=bias_sb[:, do : do + 1],
                                )
                            else:
                                # Q: scatter into per-(chunk, head) blocks with the
                                # complementary head's partitions left zero
                                for ci in range(2):
                                    for h in range(2):
                                        blk = (2 * ci + h) * CH
                                        nc.scalar.activation(
                                            out=out_sb[do][
                                                h * 64 : (h + 1) * 64,
                                                blk : blk + CH,
                                            ],
                                            in_=ps[
                                                h * 64 : (h + 1) * 64,
                                                ci * CH : (ci + 1) * CH,
                                            ],
                                            func=mybir.ActivationFunctionType.Identity,
                                            bias=bias_sb[h * 64 : (h + 1) * 64, do : do + 1],
                                        )

        # ---------------- persistent mid tiles (alloc after fm frees) ----
        mid = ctx.enter_context(tc.tile_pool(name="mid", bufs=1))
        x2_sb = [mid.tile([128, D], F32, tag=f"x2_{t}", name=f"x2_{t}")
                 for t in range(NTQ)]
        xq2_fm = mid.tile([128, ND * T_q], MD, tag="xq2fm", name="xq2fm")

        # ---------------- attention + proj ----------------
        chunks = [(0, cfg.NKTA, 0), (1, cfg.NKTB, None)]
        with tc.tile_pool(name="attn_w", bufs=1) as awp:
            # prefetch wproj while attention runs
            wproj_sb = [awp.tile([128, D], MD, tag=f"wp{d}", name=f"wp{d}") for d in range(ND)]
            for d in range(ND):
                nc.gpsimd.dma_start(
                    out=wproj_sb[d], in_=io["wproj"][d * 128 : (d + 1) * 128, :]
                )
            with tc.tile_pool(name="attn_m", bufs=1) as mp, tc.tile_pool(
                name="attn_p", bufs=4
            ) as pp, tc.tile_pool(name="attn_ps", bufs=2, space="PSUM"
            ) as aps, tc.tile_pool(name="attn_po", bufs=4, space="PSUM"
            ) as ops, tc.tile_pool(name="attn_rd", bufs=1, space="DRAM") as rdp:
                rsd = rdp.tile([2 * HP, W2], F32, tag="rsd", name="rsd")
                # masks: kti-pairs [128, 1024]; chunk A pairs 0..NKTA/2-1,
                # chunk B masked pairs at NKTA/2 ..
                mask_sb = []
                for k in range(cfg.NMASK):
                    m = mp.tile([128, 2 * W2], MD, tag=f"mask{k}")
                    nc.gpsimd.dma_start(out=m, in_=io["masks"][k, :, :])
                    mask_sb.append(m)
                for ci, nkt, _ in chunks:
                    cc = slice(ci * CH, (ci + 1) * CH)
                    npair = nkt // 2
                    for hp in range(HP):
                        po = [ops.tile([128, CH], F32, tag="po", name="po", bufs=4)
                              for _ in range(2)]
                        for kp in range(npair):
                            ps2 = aps.tile([128, 2 * W2], F32, tag="ps_s")
                            for sub in range(2):
                                kti = 2 * kp + sub
                                nc.tensor.matmul(
                                    ps2[:, sub * W2 : (sub + 1) * W2],
                                    K_sb[hp][:, kti * 128 : (kti + 1) * 128],
                                    Q_sb[hp][:, 2 * ci * CH : 2 * ci * CH + W2],
                                    start=True, stop=True,
                                )
                            if ci == 0:
                                nc.vector.tensor_add(out=ps2, in0=ps2, in1=mask_sb[kp])
                            elif 2 * kp * 128 >= cfg.klenB_pad // 2:
                                mi = cfg.NKTA // 2 + kp - cfg.NKTB // 4
                                nc.vector.tensor_add(out=ps2, in0=ps2, in1=mask_sb[mi])
                            pt = pp.tile([128, 2 * W2], MD, tag="pt")
                            nc.scalar.activation(
                                out=pt, in_=ps2,
                                func=mybir.ActivationFunctionType.Exp,
                            )
                            for sub in range(2):
                                kti = 2 * kp + sub
                                for h in range(2):
                                    hg = 2 * hp + h
                                    nc.tensor.matmul(
                                        po[h][0:65, :],
                                        V_sb[kti][:, hg * 65 : hg * 65 + 65],
                                        pt[:, sub * W2 + h * CH : sub * W2 + (h + 1) * CH],
                                        start=(kti == 0),
                                        stop=(kti == nkt - 1),
                                    )
                        # normalize + evict: den rows -> partition 0,
                        # reciprocal there, DMA-broadcast across partitions,
                        # then multiply the numerators
                        den = pp.tile([128, W2], F32, tag="densb")
                        for h in range(2):
                            nc.scalar.copy(
                                out=den[0:1, h * CH : (h + 1) * CH],
                                in_=po[h][64:65, :],
                            )
                        slot = ci * HP + hp
                        nc.sync.dma_start(
                            out=rsd[slot : slot + 1, :], in_=den[0:1, :]
                        )
                        bc = pp.tile([128, W2], F32, tag="bcsb")
                        nc.sync.dma_start(
                            out=bc,
                            in_=bass.AP(
                                tensor=rsd.tensor,
                                offset=rsd.offset + slot * W2,
                                ap=[[0, 128], [1, W2]],
                            ),
                        )
                        rc = pp.tile([128, W2], F32, tag="rcsb")
                        nc.vector.reciprocal(out=rc, in_=bc)
                        for h in range(2):
                            nc.vector.tensor_mul(
                                out=O_sb[hp][h * 64 : (h + 1) * 64, cc],
                                in0=po[h][0:64, :],
                                in1=rc[h * 64 : (h + 1) * 64, h * CH : (h + 1) * CH],
                            )

            # ---------------- proj + residual ----------------
            with tc.tile_pool(name="proj", bufs=3) as prp, tc.tile_pool(
                name="projps", bufs=4, space="PSUM"
            ) as prps:
                for qt in range(NTQ):
                    x_t = prp.tile([128, D], F32, tag="xq_res")
                    nc.sync.dma_start(
                        out=x_t, in_=io["x_q"][qt * 128 : (qt + 1) * 128, :]
                    )
                    for ch2 in range(D // 512):
                        ps = prps.tile([128, 512], F32, tag="pspr")
                        for hp in range(ND):
                            nc.tensor.matmul(
                                ps,
                                O_sb[hp][:, qt * 128 : (qt + 1) * 128],
                                wproj_sb[hp][:, ch2 * 512 : (ch2 + 1) * 512],
                                start=(hp == 0),
                                stop=(hp == ND - 1),
                            )
                        nc.vector.tensor_add(
                            out=x2_sb[qt][:, ch2 * 512 : (ch2 + 1) * 512],
                            in0=ps,
                            in1=x_t[:, ch2 * 512 : (ch2 + 1) * 512],
                        )

            # ---------------- LN2 + transpose (overlaps proj tail) --------
            with tc.tile_pool(name="ln2", bufs=3) as ln2p, tc.tile_pool(
                name="ln2st", bufs=4
            ) as ln2st, tc.tile_pool(name="tp2", bufs=2, space="PSUM") as tpp2:
                ln_transpose(None, x2_sb, NTQ, [xq2_fm], ln2p, ln2st, tpp2, "l2")

    # ---------------- fc1 + gelu + fc2 (pipelined) ----------------
    ghp = ctx.enter_context(tc.tile_pool(name="gh", bufs=1))
    gh_sb = [ghp.tile([128, T_q], MD, tag=f"gh{f}", name=f"gh{f}") for f in range(NFF)]
    with tc.tile_pool(name="fc1w", bufs=4) as f1w, tc.tile_pool(
        name="fc2w", bufs=4
    ) as f2w, tc.tile_pool(name="fc2wB", bufs=1) as f2wB, tc.tile_pool(
        name="fc2out", bufs=3
    ) as f2o, tc.tile_pool(name="fcps", bufs=3, space="PSUM"
    ) as fps, tc.tile_pool(name="fc2acc", bufs=1, space="PSUM") as f2ps:
        # preload most of sweep-1's fc2 weights during sweep 0
        NPRE = 24
        wb2_s1 = [f2wB.tile([128, 512], MD, tag=f"wfc2b{ff}", name=f"wfc2b{ff}")
                  for ff in range(NPRE)]
        for sweep in range(2):
            accs = {}
            for qt in range(NTQ):
                accs[qt] = f2ps.tile(
                    [128, 512], F32, tag=f"acc{qt}", name=f"acc{qt}"
                )
            for ff in range(NFF):
                if sweep == 0:
                    wb = f1w.tile([128, ND, 128], MD, tag="wfc1")
                    nc.gpsimd.dma_start(
                        out=wb,
                        in_=io["wfc1"][:, ff * 128 : (ff + 1) * 128].rearrange(
                            "(kt p) c -> p kt c", p=128
                        ),
                    )
                    ps = fps.tile([128, T_q], F32, tag="psf1")
                    for kt in range(ND):
                        nc.tensor.matmul(
                            ps, wb[:, kt, :],
                            xq2_fm[:, kt * 512 : (kt + 1) * 512],
                            start=(kt == 0), stop=(kt == ND - 1),
                        )
                    nc.scalar.activation(
                        out=gh_sb[ff], in_=ps,
                        func=mybir.ActivationFunctionType.Gelu,
                        bias=bfc1_sb[:, ff : ff + 1],
                    )
                if sweep == 0:
                    wb2 = f2w.tile([128, 512], MD, tag="wfc2")
                    nc.gpsimd.dma_start(
                        out=wb2, in_=io["wfc2"][ff * 128 : (ff + 1) * 128, 0:512]
                    )
                    # interleave sweep-1 weight prefetch on the same queue
                    if ff < NPRE:
                        nc.gpsimd.dma_start(
                            out=wb2_s1[ff],
                            in_=io["wfc2"][ff * 128 : (ff + 1) * 128, 512:1024],
                        )
                elif ff < NPRE:
                    wb2 = wb2_s1[ff]
                else:
                    wb2 = f2w.tile([128, 512], MD, tag="wfc2")
                    nc.gpsimd.dma_start(
                        out=wb2, in_=io["wfc2"][ff * 128 : (ff + 1) * 128, 512:1024]
                    )
                for qt in range(NTQ):
                    nc.tensor.matmul(
                        accs[qt],
                        gh_sb[ff][:, qt * 128 : (qt + 1) * 128],
                        wb2,
                        start=(ff == 0),
                        stop=(ff == NFF - 1),
                    )
            for qt in range(NTQ):
                o = f2o.tile([128, 512], F32, tag="osb")
                nc.vector.tensor_add(
                    out=o,
                    in0=accs[qt],
                    in1=x2_sb[qt][:, sweep * 512 : (sweep + 1) * 512],
                )
                nc.sync.dma_start(
                    out=io["out"][
                        qt * 128 : (qt + 1) * 128,
                        sweep * 512 : (sweep + 1) * 512,
                    ],
                    in_=o,
                )


def split_drain_waits(nc):
    """walrus CoreV3 rejects >1 sync wait on several instruction types;
    split extras into single-wait NOPs preceding the instruction on the
    same (in-order) engine."""
    idx = 0

    def fix_block(b):
        nonlocal idx
        new = []
        changed = False
        for inst in b.instructions:
            si = inst.sync_info
            if si is not None and si.on_wait and len(si.on_wait) > 1:
                waits = list(si.on_wait)
                for w in waits[:-1]:
                    idx += 1
                    nop = mybir.InstNoOp(
                        name=f"I-dsplit-{idx}",
                        sync_info=mybir.SyncInfo(on_wait=[w], on_update=[]),
                    )
                    nop.engine = inst.engine
                    new.append(nop)
                inst.sync_info = mybir.SyncInfo(
                    on_wait=[waits[-1]], on_update=list(si.on_update or [])
                )
                changed = True
            new.append(inst)
        if changed:
            b.instructions = new

    for f in nc.m.functions:
        for b in f.blocks:
            fix_block(b)


def declare_io(nc, cfg: Cfg):
    c = cfg
    WD = getattr(mybir.dt, c.mmdt)
    spec = {
        "x_kv": ([c.T_kv, c.D], F32, False),
        "x_q": ([c.T_q, c.D], F32, False),
        "wq": ([c.D, c.D], WD, False),
        "wk": ([c.D, c.D], WD, False),
        "wv": ([c.D, c.VA], WD, False),
        "bq": ([c.D], F32, False),
        "bk": ([c.D], F32, False),
        "vb": ([c.VA], F32, False),
        "wproj": ([c.D, c.D], WD, False),
        "wfc1": ([c.D, c.DFF], WD, False),
        "bfc1": ([c.DFF], F32, False),
        "wfc2": ([c.DFF, c.D], WD, False),
        "masks": ([c.NMASK, 128, 4 * c.CH], WD, False),
        "out": ([c.T_q, c.D], F32, True),
    }
    io = {}
    for name, (shape, dt, is_out) in spec.items():
        io[name] = nc.declare_dram_parameter(name, shape, dt, isOutput=is_out).ap()
    return io


def build(cfg: Cfg, split: bool = True):
    nc = bass.Bass(num_devices=8)
    io = declare_io(nc, cfg)
    with tile.TileContext(nc) as tc:
        decoder_kernel(tc, cfg, io)
    if split:
        split_drain_waits(nc)
    return nc


# ======================= host-side prep =======================


def make_masks(cfg: Cfg, qgA, qgB):
    """[NMASK, 128, 1024] kti-pair masks: 0 where key k <= query q (valid),
    else -60000. Layout per pair: [kti_even(h0 256|h1 256) | kti_odd(...)].
    Pairs 0..NKTA/2-1 are chunk A kti (0..NKTA-1); the rest are chunk B's
    masked upper-half kti (NKTB/2..NKTB-1)."""
    m = np.full((cfg.NMASK, 128, 2 * cfg.CH, 2), MASK_NEG, np.float32)

    def blk(qg, kti):
        q = qg + np.arange(cfg.CH)[None, :]
        kg = kti * 128 + np.arange(128)[:, None]
        return (kg > q).astype(np.float32) * MASK_NEG

    for kp in range(cfg.NKTA // 2):
        for sub in range(2):
            b = blk(qgA, 2 * kp + sub)
            m[kp, :, 0 : cfg.CH, sub] = b
            m[kp, :, cfg.CH : 2 * cfg.CH, sub] = b
    for i, kp in enumerate(range(cfg.NKTB // 4, cfg.NKTB // 2)):
        for sub in range(2):
            b = blk(qgB, 2 * kp + sub)
            mi = cfg.NKTA // 2 + i
            m[mi, :, 0 : cfg.CH, sub] = b
            m[mi, :, cfg.CH : 2 * cfg.CH, sub] = b
    # [NMASK,128,512,2] -> [NMASK,128,1024] with pair-sub major order
    m = m.transpose(0, 1, 3, 2).reshape(cfg.NMASK, 128, 4 * cfg.CH)
    return m.astype(np.float16)


def host_prep(cfg: Cfg, x, ln1_g, ln1_b, w_qkv, w_proj, ln2_g, ln2_b, w_fc1, w_fc2):
    """Returns (in_maps list of 8 dicts, assemble(results)->full out)."""
    D, H, DH = cfg.D, cfg.H, cfg.DH
    x = np.asarray(x, np.float32)
    B = x.shape[0]
    w_qkv = np.asarray(w_qkv, np.float32)
    bqkv = np.asarray(ln1_b, np.float32) @ w_qkv  # [3D]
    w_qkv = w_qkv * np.asarray(ln1_g, np.float32)[:, None]
    bq = bqkv[0:D] / np.sqrt(DH).astype(np.float32)
    bk = bqkv[D : 2 * D]
    bv = bqkv[2 * D : 3 * D]
    wq = w_qkv[:, 0:D] / np.sqrt(DH).astype(np.float32)
    wk = w_qkv[:, D : 2 * D]
    wv = w_qkv[:, 2 * D : 3 * D]
    wv_aug = np.zeros((D, cfg.VA), np.float32)
    vb_aug = np.zeros((cfg.VA,), np.float32)
    for h in range(H):
        wv_aug[:, h * (DH + 1) : h * (DH + 1) + DH] = wv[:, h * DH : (h + 1) * DH]
        vb_aug[h * (DH + 1) : h * (DH + 1) + DH] = bv[h * DH : (h + 1) * DH]
        vb_aug[h * (DH + 1) + DH] = 1.0
    bfc1 = np.asarray(ln2_b, np.float32) @ np.asarray(w_fc1, np.float32)
    wfc1 = np.asarray(w_fc1, np.float32) * np.asarray(ln2_g, np.float32)[:, None]

    wd = np.float32 if cfg.mmdt == "float32" else np.float16
    weights = {
        "wq": wq.astype(wd),
        "wk": wk.astype(wd),
        "wv": wv_aug.astype(wd),
        "bq": bq.astype(np.float32),
        "bk": bk.astype(np.float32),
        "vb": vb_aug.astype(np.float32),
        "wproj": np.asarray(w_proj, np.float32).astype(wd),
        "wfc1": wfc1.astype(wd),
        "bfc1": bfc1.astype(np.float32),
        "wfc2": np.asarray(w_fc2, np.float32).astype(wd),
    }

    in_maps = []
    core_rows = []
    n_j = 4  # chunk pairs per batch
    for c in range(8):
        b, j = c // n_j, c % n_j
        qgA, qgB = cfg.CH * j, cfg.CH * (2 * n_j - 1 - j)
        rows = np.r_[qgA : qgA + cfg.CH, qgB : qgB + cfg.CH]
        core_rows.append((b, rows))
        im = dict(weights)
        im["x_kv"] = np.ascontiguousarray(x[b])
        im["x_q"] = np.ascontiguousarray(x[b][rows])
        im["masks"] = make_masks(cfg, qgA, qgB).astype(wd)
        in_maps.append(im)

    def assemble(results):
        out = np.zeros((B, x.shape[1], D), np.float32)
        for c, (b, rows) in enumerate(core_rows):
            out[b][rows] = results[c]["out"]
        return out

    return in_maps, assemble


# ======================= public entry point =======================

LAST_RESULTS = {}
_CACHE = {}


def kernel(x, ln1_g, ln1_b, w_qkv, w_proj, ln2_g, ln2_b, w_fc1, w_fc2,
           _trace=False):
    """Full-input decoder block on 8 TRN2 NeuronCores; returns full output."""
    from concourse.bass_utils import run_bass_kernel_spmd

    cfg = Cfg()
    in_maps, assemble = host_prep(
        cfg, x, ln1_g, ln1_b, w_qkv, w_proj, ln2_g, ln2_b, w_fc1, w_fc2
    )
    if "nc" not in _CACHE:
        _CACHE["nc"] = build(cfg)
    res = run_bass_kernel_spmd(
        _CACHE["nc"], in_maps, core_ids=list(range(8)), trace=_trace
    )
    LAST_RESULTS["res"] = res
    return assemble(res.results)
